# revision 17
# baseline (speedup 1.0000x reference)
"""KANLinear (RBF-KAN) Trainium2 kernel.

Math (matches the reference):
  x_flat [B=8192, IN=1024]
  base   = silu(x) @ (base_w.T) + base_b
  basis[b,i,g] = exp(-(d*(x[b,i]-grid[g]))**2),  grid = linspace(-2,2,8), d = 1/(delta+1e-6)
  spline = einsum('big,oig->bo', basis, spline_w)
  out    = base + spline        [B, OUT=1024]

Implementation:
  - Data parallel over tokens: 8 cores x 1024 tokens each; weights replicated.
  - The spline contraction is a [tok, IN*G=8192] @ [8192, OUT] matmul. Per core we
    hold spline_w (transposed to [G*IN, OUT], bf16, 16MB) resident in SBUF and run
    bf16 matmuls with K accumulated in PSUM (fp32).
  - Basis tiles are produced on the fly:
      v = (x - 2g)*x          (one VectorE scalar_tensor_tensor, fp32)
      basis = Exp(-d^2*v - d^2*g^2)   (one ScalarE activation, bf16 out)
    which equals exp(-d^2 (x-g)^2) exactly.
  - silu(x) is computed as x*(1+tanh(x/2)) (tanh lives in the same ACT table set
    as exp, avoiding table switches); the 0.5 factor is folded into base_w host-side.
  - base_b is added via a K=1 rank-1 matmul (ones row x bias row).
  - Layout: out[tokens(part), out(free)] so the result DMAs out contiguously.
"""

import os
import sys

os.environ.setdefault("MYCRO_LOCAL_CACHE", "1")
for _p in ("/opt/trn_rl_repo", "/root/.axon_site/_ro/trn_rl_repo"):
    if os.path.isdir(_p) and _p not in sys.path:
        sys.path.insert(0, _p)

import numpy as np
import ml_dtypes

IN_F = 1024
OUT_F = 1024
G = 8
GRID_LO, GRID_HI = -2.0, 2.0
NCORES = 8
TOK = 8192
TCORE = TOK // NCORES   # 1024 tokens per core
NG = 2                  # token groups per core
GTOK = TCORE // NG      # 512 tokens per group
MT = GTOK // 128        # 4 psum m-tiles (128 tokens) per group
KS = G * (IN_F // 128)  # 64 spline k-tiles
KB = IN_F // 128        # 8 base k-tiles

_DELTA = float((GRID_HI - GRID_LO) / (G - 1))
_D = 1.0 / (_DELTA + 1e-6)
# match jax's f32 linspace values
_GRID = np.linspace(GRID_LO, GRID_HI, G, dtype=np.float32).astype(np.float64)

TRACE = False
LAST_RESULT = None
_NC_CACHE = None


def build_nc(reps=1):
    from concourse import bacc
    import concourse.mybir as mybir
    import concourse.tile as tile

    F32 = mybir.dt.float32
    BF16 = mybir.dt.bfloat16
    Alu = mybir.AluOpType
    Act = mybir.ActivationFunctionType

    nc = bacc.Bacc("TRN2", target_bir_lowering=False)
    xg_d = nc.dram_tensor("xg", [NG, 128, KB, GTOK], F32, kind="ExternalInput")
    spl_d = nc.dram_tensor("spline", [KS * 128, OUT_F], BF16, kind="ExternalInput")
    bw_d = nc.dram_tensor("basew", [IN_F, OUT_F], BF16, kind="ExternalInput")
    bb_d = nc.dram_tensor("brow", [1, OUT_F], BF16, kind="ExternalInput")
    out_d = nc.dram_tensor("out", [TCORE, OUT_F], F32, kind="ExternalOutput")

    d2 = _D * _D

    # Register const APs for the per-grid Exp biases (activation() requires a
    # pre-registered [128,1] const tensor for non-trivial float biases).
    def register_const_ap(value):
        t = nc.alloc_sbuf_tensor(f"const-bias-{value}", [128, 1], F32)
        nc.gpsimd.memset(t.ap(), value)
        nc.const_aps.aps[(F32, value)] = t.ap()

    def exp_bias(g):
        gval = float(_GRID[g])
        return float(-d2 * gval * gval)

    for value in sorted({exp_bias(g) for g in range(G)}):
        register_const_ap(value)
    nc.all_engine_barrier()

    with tile.TileContext(nc) as tc:
        with (
            tc.tile_pool(name="const", bufs=1) as cpool,
            tc.tile_pool(name="xg", bufs=2) as xpool,
            tc.tile_pool(name="silu", bufs=1) as spool,
            tc.tile_pool(name="tanh", bufs=1) as tpool,
            tc.tile_pool(name="v", bufs=2) as vpool,
            tc.tile_pool(name="basis", bufs=3) as bpool,
            tc.tile_pool(name="osb", bufs=3) as opool,
            tc.tile_pool(name="psum", bufs=4, space="PSUM") as ppool,
        ):
            spl_sb = cpool.tile([128, KS, OUT_F], BF16)
            bw_sb = cpool.tile([128, KB, OUT_F], BF16)
            ones_sb = cpool.tile([1, 128], BF16)
            brow_sb = cpool.tile([1, OUT_F], BF16)
            spl_view = spl_d[:].rearrange("(k p) n -> p k n", p=128)
            bw_view = bw_d[:].rearrange("(k p) n -> p k n", p=128)

            pending = []  # psum tiles of the previous group awaiting eviction

            def emit_evictions():
                for idx, (ps_t, mg) in enumerate(pending):
                    o = opool.tile([128, OUT_F], F32, tag="osb", name=f"o_{mg}")
                    if idx % 2 == 0:
                        nc.vector.tensor_copy(o[:], ps_t[:])
                    else:
                        nc.scalar.copy(o[:], ps_t[:])
                    nc.sync.dma_start(out_d[mg * 128:(mg + 1) * 128, :], o[:])
                pending.clear()

            for rep in range(reps):
              for grp in range(NG):
                xg = xpool.tile([128, KB, GTOK], F32, tag="xg", name=f"xg_r{rep}g{grp}")
                if grp == 0:
                    # interleave the x block and the first spline k-tiles so
                    # the PE can start within a few us; then the bulk loads
                    nc.sync.dma_start(xg[:, 0:4, :], xg_d[grp, :, 0:4, :])
                    nc.sync.dma_start(spl_sb[:, 0:1, :], spl_view[:, 0:1, :])
                    nc.sync.dma_start(xg[:, 4:8, :], xg_d[grp, :, 4:8, :])
                    nc.sync.dma_start(spl_sb[:, 1:8, :], spl_view[:, 1:8, :])
                    for c in range(1, 8):
                        nc.sync.dma_start(
                            spl_sb[:, c * 8:(c + 1) * 8, :],
                            spl_view[:, c * 8:(c + 1) * 8, :],
                        )
                    nc.sync.dma_start(bw_sb[:], bw_view[:])
                    nc.vector.memset(ones_sb[:], 1.0)
                    nc.sync.dma_start(brow_sb[:], bb_d[:])
                else:
                    nc.sync.dma_start(xg[:], xg_d[grp, :, :, :])
                silu = spool.tile([128, KB, GTOK], BF16)
                ps = [
                    ppool.tile([128, OUT_F], F32, tag="ps", name=f"ps_g{grp}m{m}")
                    for m in range(MT)
                ]

                for k in range(KS):
                    g, i = divmod(k, KB)
                    gval = float(_GRID[g])
                    v = vpool.tile([128, GTOK], F32)
                    nc.vector.scalar_tensor_tensor(
                        v[:], xg[:, i, :], -2.0 * gval, xg[:, i, :],
                        op0=Alu.add, op1=Alu.mult,
                    )
                    basis = bpool.tile([128, GTOK], BF16)
                    nc.scalar.activation(
                        basis[:], v[:], Act.Exp,
                        bias=exp_bias(k // KB), scale=float(-d2),
                    )
                    for m in range(MT):
                        lhsT = basis[:, m * 128:(m + 1) * 128]
                        for n in range(2):
                            nc.tensor.matmul(
                                ps[m][:, n * 512:(n + 1) * 512],
                                lhsT,
                                spl_sb[:, k, n * 512:(n + 1) * 512],
                                start=(k == 0), stop=False,
                            )
                    if k == 4 and pending:
                        emit_evictions()
                    if k == 8:
                        # silu2 = x*(1+tanh(x/2)) = 2*silu(x); 0.5 folded into basew
                        for i2 in range(KB):
                            t = tpool.tile([128, GTOK], F32)
                            nc.scalar.activation(t[:], xg[:, i2, :], Act.Tanh, scale=0.5)
                            nc.vector.scalar_tensor_tensor(
                                silu[:, i2, :], t[:], 1.0, xg[:, i2, :],
                                op0=Alu.add, op1=Alu.mult,
                            )

                last = grp == NG - 1
                if not last:
                    # base phase, m-interleaved; bias via rank-1 ones x brow
                    for kb in range(KB):
                        for m in range(MT):
                            lhsT = silu[:, kb, m * 128:(m + 1) * 128]
                            for n in range(2):
                                nc.tensor.matmul(
                                    ps[m][:, n * 512:(n + 1) * 512],
                                    lhsT,
                                    bw_sb[:, kb, n * 512:(n + 1) * 512],
                                    start=False, stop=False,
                                )
                    for m in range(MT):
                        for n in range(2):
                            nc.tensor.matmul(
                                ps[m][:, n * 512:(n + 1) * 512],
                                ones_sb[0:1, :],
                                brow_sb[0:1, n * 512:(n + 1) * 512],
                                start=False, stop=True,
                            )
                        pending.append((ps[m], grp * MT + m))
                else:
                    # last group: finish one m-tile at a time so evictions
                    # overlap the remaining base matmuls instead of the tail
                    for m in range(MT):
                        for kb in range(KB):
                            lhsT = silu[:, kb, m * 128:(m + 1) * 128]
                            for n in range(2):
                                nc.tensor.matmul(
                                    ps[m][:, n * 512:(n + 1) * 512],
                                    lhsT,
                                    bw_sb[:, kb, n * 512:(n + 1) * 512],
                                    start=False, stop=False,
                                )
                        for n in range(2):
                            nc.tensor.matmul(
                                ps[m][:, n * 512:(n + 1) * 512],
                                ones_sb[0:1, :],
                                brow_sb[0:1, n * 512:(n + 1) * 512],
                                start=False, stop=True,
                            )
                        pending.append((ps[m], grp * MT + m))
                        emit_evictions()
            emit_evictions()

    nc.compile()
    return nc


def _host_prep(x, base_w, base_b, spline_w):
    x = np.asarray(x, dtype=np.float32)
    base_w = np.asarray(base_w, dtype=np.float32)
    base_b = np.asarray(base_b, dtype=np.float32)
    spline_w = np.asarray(spline_w, dtype=np.float32)

    x_flat = np.ascontiguousarray(x.reshape(TOK, IN_F))
    # [OUT, IN, G] -> [G, IN, OUT] -> [G*IN, OUT]; row r = g*IN + i
    spl = np.ascontiguousarray(spline_w.transpose(2, 1, 0).reshape(G * IN_F, OUT_F))
    spl = spl.astype(ml_dtypes.bfloat16)
    bw = np.ascontiguousarray(0.5 * base_w.T).astype(ml_dtypes.bfloat16)
    brow = np.ascontiguousarray(base_b.reshape(1, OUT_F)).astype(ml_dtypes.bfloat16)

    in_maps = []
    for c in range(NCORES):
        shard = x_flat[c * TCORE:(c + 1) * TCORE, :]   # [tok, in]
        xT = shard.T                                    # [in, tok]
        # [in, tok] -> [i, p, grp, t] -> [grp, p, i, t]
        xg = np.ascontiguousarray(
            xT.reshape(KB, 128, NG, GTOK).transpose(2, 1, 0, 3)
        )
        in_maps.append({"xg": xg, "spline": spl, "basew": bw, "brow": brow})
    return in_maps


def kernel(x, base_w, base_b, spline_w):
    global _NC_CACHE, LAST_RESULT
    from concourse.bass_utils import run_bass_kernel_spmd

    in_maps = _host_prep(x, base_w, base_b, spline_w)
    if _NC_CACHE is None:
        _NC_CACHE = build_nc()
    res = run_bass_kernel_spmd(
        _NC_CACHE, in_maps, core_ids=list(range(NCORES)), trace=TRACE
    )
    LAST_RESULT = res
    outs = [np.asarray(r["out"]) for r in res.results]
    full = np.concatenate(outs, axis=0)  # [8192, 1024]
    return full.reshape(4, 2048, OUT_F)
